# revision 19
# baseline (speedup 1.0000x reference)
"""Trainium2 Bass kernel for nn_LinearEncoder (gnn_message_passing).

Reference computes, for N=512 nodes with n_in = n_out = 256:
    i, j = triu_indices(N, k=1)
    edges = concat([x[i], x[j]], -1)            # [E, 512]
    h = edges @ W.T + b                         # [E, 256]
    out[i, j] = h ; out = out + out.T           # [N, N, 256], 0 diagonal

Key algebraic identity: with W = [W1 | W2],
    h(i, j) = A[i] + B[j] + b,   A = x @ W1.T,  B = x @ W2.T
so the full output is
    out[i, j] = A[min(i,j)] + B'[max(i,j)]      (B' = B + b), 0 on diagonal.

Sharding: output rows split across 8 cores (64 rows each), one SPMD
program.  Core k receives x pre-rotated by its row base
(x_rot[t] = x[(base+t) % 512]) so the triangular "diagonal block" sits at
local columns s in [0, 64) on every core; region selection (A vs B')
enters only through small 0/1 mask *inputs*.

Per row-pair rp (rows r0 = 2rp, r0+1), the device computes:
  - three 128-wide column blocks:  PSUM = masks.T @ row-table (bf16 hi+lo
    split, exact to ~2^-17), run CONCURRENTLY on the PE via distinct
    row-groups (tile_position), then evacuated by VectorE as
    sbuf = PSUM + ColTable_f32 (column terms exact fp32);
  - block0 upper half: same masked-broadcast + DVE fold;
  - the triangular diagonal block: two constant masked-selection matmul
    pairs (including the exact-zero diagonal), evacuated by ScalarE.
DMA streams ~33.5 MB/core of output to HBM — the roofline.
"""

import os
import sys

for _p in ("/opt/trn_rl_repo", "/root/.axon_site/_ro/trn_rl_repo"):
    if os.path.isdir(_p) and _p not in sys.path:
        sys.path.insert(0, _p)

import numpy as np
import ml_dtypes

import concourse.bass as bass
import concourse.bacc as bacc
import concourse.mybir as mybir
import concourse.tile as tile
from concourse.bass_utils import run_bass_kernel_spmd

N = 512
CH = 256          # n_out
NIN = 256         # n_in
NCORES = 8
RB = N // NCORES  # 64 rows per core
F32 = mybir.dt.float32
BF16 = mybir.dt.bfloat16
BF16NP = ml_dtypes.bfloat16


# --------------------------------------------------------------------------
# host-side constant builders
# --------------------------------------------------------------------------

def _masks_RL(k: int):
    """R/L region indicators over local columns s for core k."""
    base = RB * k
    wrap = N - base  # columns s >= wrap hold wrapped (j < base) entries
    s = np.arange(N)
    R = ((s >= 64) & (s < wrap)).astype(np.float32)
    L = (s >= wrap).astype(np.float32)
    return R, L


def _diag_consts():
    """Constant masked-selection weights for the 64x64 diagonal blocks.

    For row-pair rp, output column m = q*64 + s (q in {0,1}, s in [0,64)),
    with r_q = 2*rp + q and rhs = [A_rot[0:64] ; B'_rot[0:64]] (K = 128):
      L side (s < r_q):  value = A_rot[s] + B'_rot[r_q]
      R side (s > r_q):  value = B'_rot[s] + A_rot[r_q]
      s == r_q: all weights zero -> exact 0 output.
    """
    dl = np.zeros((128, 32 * 128), np.float32)
    dr = np.zeros((128, 32 * 128), np.float32)
    for rp in range(32):
        for q in range(2):
            r_q = 2 * rp + q
            for s in range(64):
                m = rp * 128 + q * 64 + s
                if s < r_q:
                    dl[s, m] = 1.0            # A_rot[s]
                    dl[64 + r_q, m] = 1.0     # B'_rot[r_q]
                elif s > r_q:
                    dr[64 + s, m] = 1.0       # B'_rot[s]
                    dr[r_q, m] = 1.0          # A_rot[r_q]
    return dl, dr


def _shared_inputs(W: np.ndarray, b: np.ndarray):
    W = np.asarray(W, np.float32)
    b = np.asarray(b, np.float32)
    dl, dr = _diag_consts()
    w12 = np.concatenate(
        [np.ascontiguousarray(W[:, :NIN].T), np.ascontiguousarray(W[:, NIN:].T)],
        axis=1)                                     # [in, 2*out] = [A | B]
    b2 = np.concatenate([np.zeros(CH, np.float32), b]).reshape(1, 2 * CH)
    i64p = np.concatenate([np.eye(64, dtype=np.float32)] * 2, axis=1)
    return {
        "w12t": w12,
        "b2_row": b2,
        "diag_l": dl.astype(BF16NP),
        "diag_r": dr.astype(BF16NP),
        "i64p": i64p.astype(BF16NP),
    }


def _core_inputs(x: np.ndarray, k: int):
    x = np.asarray(x, np.float32)
    base = RB * k
    x_rot = np.roll(x, -base, axis=0)
    R, L = _masks_RL(k)

    cm = np.zeros((128, 8), np.float32)
    for t in range(4):
        cm[:, t] = R[128 * t:128 * (t + 1)]
        cm[:, 4 + t] = L[128 * t:128 * (t + 1)]

    # One [128, 512] lhsT tensor: the masked-broadcast weights for the
    # three main column blocks live in PE row-groups 0/1/2 (partitions
    # 0-3, 32-35, 64-67; rows = R, L, R, L over hi/lo flat tables) and
    # block0-upper's K=8 weights in row-group 3 (partitions 96-103) —
    # the four small-K matmuls then run concurrently on the PE.
    wm4 = np.stack([R, L, R, L])                        # [4, 512]
    wm0 = np.zeros((8, 128), np.float32)
    p = np.arange(64)
    wm0[0, :64] = R[64 + p]
    wm0[1, :64] = L[64 + p]
    wm0[2, :64] = R[64 + p]
    wm0[3, :64] = L[64 + p]
    wm0[4, 64:] = R[64 + p]
    wm0[5, 64:] = L[64 + p]
    wm0[6, 64:] = R[64 + p]
    wm0[7, 64:] = L[64 + p]
    wmbig = np.zeros((128, 512), np.float32)
    for gp in (0, 32, 64):
        wmbig[gp:gp + 4, :] = wm4
    wmbig[96:104, 0:128] = wm0
    return {
        "xt_rot": np.ascontiguousarray(x_rot.T),  # [in=256, node=512]
        "cm": cm,
        "wm": wmbig.astype(BF16NP),
    }


# --------------------------------------------------------------------------
# device program
# --------------------------------------------------------------------------

_PROGRAM = None


def _build_program() -> bass.Bass:
    nc = bacc.Bacc()
    f32 = F32
    npad = 68  # padded flat scratch rows

    # ---- dram tensors -----------------------------------------------------
    xt_rot = nc.dram_tensor("xt_rot", [NIN, N], f32, kind="ExternalInput")
    w12t = nc.dram_tensor("w12t", [NIN, 2 * CH], f32, kind="ExternalInput")
    b2_row = nc.dram_tensor("b2_row", [1, 2 * CH], f32, kind="ExternalInput")
    cm = nc.dram_tensor("cm", [128, 8], f32, kind="ExternalInput")
    d_wm = nc.dram_tensor("wm", [128, N], BF16, kind="ExternalInput")
    d_dl = nc.dram_tensor("diag_l", [128, 32 * 128], BF16, kind="ExternalInput")
    d_dr = nc.dram_tensor("diag_r", [128, 32 * 128], BF16, kind="ExternalInput")
    d_i64p = nc.dram_tensor("i64p", [64, 128], BF16, kind="ExternalInput")

    # DMA-native contiguous layouts; the host unpicks them (free).
    # slab_m[3g + J-1, p, (sub, q, ch)] = value(row 8g+2sub+q, s = 128J+p)
    # out0d/u[g, q*64+s, (sub, ch)]    = diag/upper block values
    slab_m = nc.dram_tensor("slab_m", [24, 128, 2048], f32,
                            kind="ExternalOutput")
    out0d = nc.dram_tensor("out0d", [8, 128, 1024], f32, kind="ExternalOutput")
    out0u = nc.dram_tensor("out0u", [8, 128, 1024], f32, kind="ExternalOutput")

    with tile.TileContext(nc) as tc:
        with (
            tc.tile_pool(name="const", bufs=1) as cpool,
            tc.tile_pool(name="tmp", bufs=3) as tpool,
            tc.tile_pool(name="psA", bufs=4, space="PSUM") as psA,
            tc.tile_pool(name="ps0", bufs=4, space="PSUM") as ps0,
            tc.tile_pool(name="stM", bufs=6) as stM,
            tc.tile_pool(name="st0", bufs=4) as st0,
        ):
            # ---- load inputs ---------------------------------------------
            def load(dram, shape, dtype, tag):
                t = cpool.tile(shape, dtype, tag=tag)
                nc.sync.dma_start(out=t[:], in_=dram[:])
                return t

            xt0 = load(xt_rot[0:128, :], [128, N], f32, "xt0")
            xt1 = load(xt_rot[128:256, :], [128, N], f32, "xt1")
            w12a = load(w12t[0:128, :], [128, 2 * CH], f32, "w12a")
            w12b = load(w12t[128:256, :], [128, 2 * CH], f32, "w12b")
            b2t = load(b2_row, [1, 2 * CH], f32, "b2t")
            cmt = load(cm, [128, 8], f32, "cmt")
            wmt = load(d_wm, [128, N], BF16, "wmt")
            i64pt = load(d_i64p, [64, 128], BF16, "i64pt")
            dlt = cpool.tile([128, 32 * 128], BF16, tag="dlt")
            nc.gpsimd.dma_start(out=dlt[:], in_=d_dl[:])
            drt = cpool.tile([128, 32 * 128], BF16, tag="drt")
            nc.gpsimd.dma_start(out=drt[:], in_=d_dr[:])

            ones1 = cpool.tile([1, 128], f32, tag="ones1")
            nc.vector.memset(ones1[:], 1.0)

            # ---- phase 1: tables [A | B'] (one [128, 512] psum per s) ----
            A_t, Bp_t = [], []
            for s in range(4):
                pa = psA.tile([128, 2 * CH], f32, tag="pj", name=f"ptb{s}")
                mmd = nc.tensor.matmul
                mmd(pa[:], xt0[:, 128 * s:128 * (s + 1)], w12a[:],
                    start=True, stop=False)
                mmd(pa[:], xt1[:, 128 * s:128 * (s + 1)], w12b[:],
                    start=False, stop=False)
                mmd(pa[:], ones1[:], b2t[:], start=False, stop=True)
                comb = cpool.tile([128, 2 * CH], f32, tag=f"AB{s}")
                nc.vector.tensor_copy(out=comb[:], in_=pa[:])
                A_t.append(comb[:, 0:CH])
                Bp_t.append(comb[:, CH:2 * CH])

            # ---- phase 1b: mixed column tables Cmix = R*B' + L*A (f32) ---
            Cmix = []
            for s in range(4):
                t1 = tpool.tile([128, CH], f32, tag="t1")
                nc.vector.tensor_scalar_mul(t1[:], Bp_t[s], cmt[:, s:s + 1])
                t2 = tpool.tile([128, CH], f32, tag="t2")
                nc.vector.tensor_scalar_mul(t2[:], A_t[s], cmt[:, 4 + s:5 + s])
                cx = cpool.tile([128, CH], f32, tag=f"C{s}")
                nc.vector.tensor_add(cx[:], t1[:], t2[:])
                Cmix.append(cx)

            # duplicated f32 column tables for the r-paired main tiles
            CD = {}
            for s in (1, 2, 3):
                dup = cpool.tile([128, 2 * CH], f32, tag=f"CD{s}")
                nc.vector.tensor_copy(out=dup[:, 0:CH], in_=Cmix[s][:])
                nc.scalar.copy(out=dup[:, CH:2 * CH], in_=Cmix[s][:])
                CD[s] = dup


            def hi_lo(src_ap, tag):
                """split a f32 [128, W] AP into bf16 hi + lo tiles."""
                wdt = src_ap.shape[-1]
                hi = cpool.tile([128, wdt], BF16, tag=f"{tag}h")
                nc.vector.tensor_copy(out=hi[:], in_=src_ap)
                h32 = tpool.tile([128, wdt], f32, tag="h32")
                nc.vector.tensor_copy(out=h32[:], in_=hi[:])
                d = tpool.tile([128, wdt], f32, tag="d32")
                nc.vector.tensor_sub(d[:], src_ap, h32[:])
                lo = cpool.tile([128, wdt], BF16, tag=f"{tag}l")
                nc.vector.tensor_copy(out=lo[:], in_=d[:])
                return hi, lo

            ah, al = hi_lo(A_t[0], "a0")
            bh, bl = hi_lo(Bp_t[0], "b0")
            c0h, c0l = hi_lo(Cmix[0][:], "c0")
            cuh = cpool.tile([64, CH], BF16, tag="cuh")
            cul = cpool.tile([64, CH], BF16, tag="cul")
            nc.sync.dma_start(out=cuh[:], in_=c0h[64:128, :])
            nc.sync.dma_start(out=cul[:], in_=c0l[64:128, :])
            # diag combined rhs [A_rot[0:64] ; B'_rot[0:64]] (hi / lo)
            dcb_h = cpool.tile([128, CH], BF16, tag="dcbh")
            dcb_l = cpool.tile([128, CH], BF16, tag="dcbl")
            nc.vector.tensor_copy(out=dcb_h[0:64, :], in_=ah[0:64, :])
            nc.vector.tensor_copy(out=dcb_l[0:64, :], in_=al[0:64, :])
            nc.sync.dma_start(out=dcb_h[64:128, :], in_=bh[0:64, :])
            nc.sync.dma_start(out=dcb_l[64:128, :], in_=bl[0:64, :])
            # flat row tables: direct SBUF->SBUF flatten into partitions
            # 0-7, then replicated to partition groups 32/64/96 (walrus
            # requires rhs to start at the same partition as the weights).
            rp4 = cpool.tile([104, 64 * CH], BF16, tag="rp4")
            nc.vector.memset(rp4[0:8, 63 * CH:64 * CH], 0.0)
            for i, t in enumerate((ah, bh, al, bl)):
                nc.sync.dma_start(out=rp4[i:i + 1, :], in_=t[0:64, :])
                nc.sync.dma_start(out=rp4[4 + i:5 + i, 0:63 * CH],
                                  in_=t[1:64, :])
            for gp in (32, 64, 96):
                nc.sync.dma_start(out=rp4[gp:gp + 8, :], in_=rp4[0:8, :])

            # ---- phase 2: main loop --------------------------------------
            for g in range(8):
                sM = {J: stM.tile([128, 4 * 512], f32, tag="sm",
                                  name=f"sm_{g}_{J}")
                      for J in (1, 2, 3)}
                s0d = st0.tile([128, 4 * CH], f32, tag="s0")
                s0u = st0.tile([128, 4 * CH], f32, tag="s0")
                for sub in range(4):
                    rp = 4 * g + sub
                    off = 2 * rp * CH
                    # four small-K masked-broadcast matmuls in distinct PE
                    # row-groups -> concurrent execution.
                    pj = {}
                    for J in (1, 2, 3):
                        gp = 32 * (J - 1)
                        p = psA.tile([128, 512], f32, tag="pj",
                                     name=f"pj_{rp}_{J}")
                        nc.tensor.matmul(
                            p[:], wmt[gp:gp + 4, 128 * J:128 * (J + 1)],
                            rp4[gp:gp + 4, off:off + 512],
                            start=True, stop=True, tile_position=(gp, 0))
                        pj[J] = p
                    pu = ps0.tile([128, CH], f32, tag="p0", name=f"pu_{rp}")
                    nc.tensor.matmul(pu[:], i64pt[:], cuh[:],
                                     start=True, stop=False)
                    nc.tensor.matmul(pu[:], i64pt[:], cul[:],
                                     start=False, stop=False)
                    nc.tensor.matmul(
                        pu[:], wmt[96:104, 0:128],
                        rp4[96:104, off:off + CH],
                        start=False, stop=True, tile_position=(96, 0))
                    # diagonal block (s in [0,64)), rows r0, r0+1
                    pd = ps0.tile([128, CH], f32, tag="p0", name=f"pd_{rp}")
                    dl_sl = dlt[:, 128 * rp:128 * (rp + 1)]
                    dr_sl = drt[:, 128 * rp:128 * (rp + 1)]
                    nc.tensor.matmul(pd[:], dl_sl, dcb_h[:],
                                     start=True, stop=False)
                    nc.tensor.matmul(pd[:], dl_sl, dcb_l[:],
                                     start=False, stop=False)
                    nc.tensor.matmul(pd[:], dr_sl, dcb_h[:],
                                     start=False, stop=False)
                    nc.tensor.matmul(pd[:], dr_sl, dcb_l[:],
                                     start=False, stop=True)
                    # evacuation: VectorE folds the f32 column tables in;
                    # ScalarE evacuates the diagonal block.
                    for J in (1, 2, 3):
                        nc.vector.tensor_add(
                            sM[J][:, 512 * sub:512 * (sub + 1)],
                            pj[J][:], CD[J][:])
                    nc.scalar.copy(out=s0u[:, CH * sub:CH * (sub + 1)],
                                   in_=pu[:])
                    nc.scalar.copy(out=s0d[:, CH * sub:CH * (sub + 1)],
                                   in_=pd[:])
                    if sub in (1, 3):
                        h = (sub - 1) // 2
                        hs, he = 1024 * h, 1024 * (h + 1)
                        for J in (1, 2):
                            nc.sync.dma_start(
                                out=slab_m[3 * g + J - 1][:, hs:he],
                                in_=sM[J][:, hs:he])
                        nc.scalar.dma_start(
                            out=slab_m[3 * g + 2][:, hs:he],
                            in_=sM[3][:, hs:he])
                        nc.scalar.dma_start(
                            out=out0u[g][:, 512 * h:512 * (h + 1)],
                            in_=s0u[:, 512 * h:512 * (h + 1)])
                        nc.scalar.dma_start(
                            out=out0d[g][:, 512 * h:512 * (h + 1)],
                            in_=s0d[:, 512 * h:512 * (h + 1)])

    nc.compile()
    return nc


def _program() -> bass.Bass:
    global _PROGRAM
    if _PROGRAM is None:
        _PROGRAM = _build_program()
    return _PROGRAM


# --------------------------------------------------------------------------
# host entry point
# --------------------------------------------------------------------------

def _assemble(results):
    """8 per-core result dicts -> full [512, 512, 256] output."""
    out = np.empty((N, N, CH), np.float32)
    for k in range(NCORES):
        r = results[k]
        slab = np.empty((RB, N, CH), np.float32)
        # out0d/u: [g, q*64+s, (sub, ch)] -> rows 8g+2sub+q, cols s / 64+s
        d = np.asarray(r["out0d"]).reshape(8, 2, 64, 4, CH)
        slab[:, 0:64, :] = d.transpose(0, 3, 1, 2, 4).reshape(RB, 64, CH)
        u = np.asarray(r["out0u"]).reshape(8, 2, 64, 4, CH)
        slab[:, 64:128, :] = u.transpose(0, 3, 1, 2, 4).reshape(RB, 64, CH)
        # slab_m: [3g+J-1, p, (sub, q, ch)] -> rows 8g+2sub+q, col 128J+p
        m = np.asarray(r["slab_m"]).reshape(8, 3, 128, 4, 2, CH)
        slab[:, 128:512, :] = (
            m.transpose(0, 3, 4, 1, 2, 5).reshape(RB, 384, CH))
        base = RB * k
        out[base:base + RB] = np.roll(slab, base, axis=1)
    return out


def build_in_maps(x, W, b):
    shared = _shared_inputs(W, b)
    return [dict(shared, **_core_inputs(x, k)) for k in range(NCORES)]


def kernel(x, W, b):
    nc = _program()
    in_maps = build_in_maps(x, W, b)
    res = run_bass_kernel_spmd(nc, in_maps, core_ids=list(range(NCORES)))
    return _assemble(res.results)


# revision 21
# speedup vs baseline: 1.0711x; 1.0711x over previous
"""Trainium2 Bass kernel for nn_LinearEncoder (gnn_message_passing).

Reference computes, for N=512 nodes with n_in = n_out = 256:
    i, j = triu_indices(N, k=1)
    edges = concat([x[i], x[j]], -1)            # [E, 512]
    h = edges @ W.T + b                         # [E, 256]
    out[i, j] = h ; out = out + out.T           # [N, N, 256], 0 diagonal

Key algebraic identity: with W = [W1 | W2],
    h(i, j) = A[i] + B[j] + b,   A = x @ W1.T,  B = x @ W2.T
so the full output is
    out[i, j] = A[min(i,j)] + B'[max(i,j)]      (B' = B + b), 0 on diagonal.

Sharding: output rows split across 8 cores (64 rows each), one SPMD
program.  Core k receives x pre-rotated by its row base
(x_rot[t] = x[(base+t) % 512]) so the triangular "diagonal block" sits at
local columns s in [0, 64) on every core; region selection (A vs B')
enters only through small 0/1 mask *inputs*.

Per row-pair rp (rows r0 = 2rp, r0+1), the device computes:
  - three 128-wide column blocks:  PSUM = masks.T @ row-table (bf16 hi+lo
    split, exact to ~2^-17), run CONCURRENTLY on the PE via distinct
    row-groups (tile_position), then evacuated by VectorE as
    sbuf = PSUM + ColTable_f32 (column terms exact fp32);
  - block0 upper half: same masked-broadcast + DVE fold;
  - the triangular diagonal block: two constant masked-selection matmul
    pairs (including the exact-zero diagonal), evacuated by ScalarE.
DMA streams ~33.5 MB/core of output to HBM — the roofline.
"""

import os
import sys

for _p in ("/opt/trn_rl_repo", "/root/.axon_site/_ro/trn_rl_repo"):
    if os.path.isdir(_p) and _p not in sys.path:
        sys.path.insert(0, _p)

import numpy as np
import ml_dtypes

import concourse.bass as bass
import concourse.bacc as bacc
import concourse.mybir as mybir
import concourse.tile as tile
from concourse.bass_utils import run_bass_kernel_spmd

N = 512
CH = 256          # n_out
NIN = 256         # n_in
NCORES = 8
RB = N // NCORES  # 64 rows per core
F32 = mybir.dt.float32
BF16 = mybir.dt.bfloat16
BF16NP = ml_dtypes.bfloat16


# --------------------------------------------------------------------------
# host-side constant builders
# --------------------------------------------------------------------------

def _masks_RL(k: int):
    """R/L region indicators over local columns s for core k."""
    base = RB * k
    wrap = N - base  # columns s >= wrap hold wrapped (j < base) entries
    s = np.arange(N)
    R = ((s >= 64) & (s < wrap)).astype(np.float32)
    L = (s >= wrap).astype(np.float32)
    return R, L


def _diag_consts():
    """Constant masked-selection weights for the 64x64 diagonal blocks.

    For row-pair rp, output column m = q*64 + s (q in {0,1}, s in [0,64)),
    with r_q = 2*rp + q and rhs = [A_rot[0:64] ; B'_rot[0:64]] (K = 128):
      L side (s < r_q):  value = A_rot[s] + B'_rot[r_q]
      R side (s > r_q):  value = B'_rot[s] + A_rot[r_q]
      s == r_q: all weights zero -> exact 0 output.
    """
    dl = np.zeros((128, 32 * 128), np.float32)
    dr = np.zeros((128, 32 * 128), np.float32)
    for rp in range(32):
        for q in range(2):
            r_q = 2 * rp + q
            for s in range(64):
                m = rp * 128 + q * 64 + s
                if s < r_q:
                    dl[s, m] = 1.0            # A_rot[s]
                    dl[64 + r_q, m] = 1.0     # B'_rot[r_q]
                elif s > r_q:
                    dr[64 + s, m] = 1.0       # B'_rot[s]
                    dr[r_q, m] = 1.0          # A_rot[r_q]
    return dl, dr


def _shared_inputs(W: np.ndarray, b: np.ndarray):
    W = np.asarray(W, np.float32)
    b = np.asarray(b, np.float32)
    dl, dr = _diag_consts()
    w12 = np.concatenate(
        [np.ascontiguousarray(W[:, :NIN].T), np.ascontiguousarray(W[:, NIN:].T)],
        axis=1)                                     # [in, 2*out] = [A | B]
    b2 = np.concatenate([np.zeros(CH, np.float32), b]).reshape(1, 2 * CH)
    return {
        "w12t": w12,
        "b2_row": b2,
        "diag_l": dl.astype(BF16NP),
        "diag_r": dr.astype(BF16NP),
    }


def _core_inputs(x: np.ndarray, k: int):
    x = np.asarray(x, np.float32)
    base = RB * k
    x_rot = np.roll(x, -base, axis=0)
    R, L = _masks_RL(k)

    cm = np.zeros((128, 8), np.float32)
    for t in range(4):
        cm[:, t] = R[128 * t:128 * (t + 1)]
        cm[:, 4 + t] = L[128 * t:128 * (t + 1)]

    # One [128, 512] lhsT tensor: the masked-broadcast weights for the
    # three main column blocks live in PE row-groups 0/1/2 (partitions
    # 0-3, 32-35, 64-67; rows = R, L, R, L over hi/lo flat tables) and
    # block0-upper's K=8 weights in row-group 3 (partitions 96-103) —
    # the four small-K matmuls then run concurrently on the PE.
    wm4 = np.stack([R, L, R, L])                        # [4, 512]
    wm0 = np.zeros((8, 128), np.float32)
    p = np.arange(64)
    wm0[0, :64] = R[64 + p]
    wm0[1, :64] = L[64 + p]
    wm0[2, :64] = R[64 + p]
    wm0[3, :64] = L[64 + p]
    wm0[4, 64:] = R[64 + p]
    wm0[5, 64:] = L[64 + p]
    wm0[6, 64:] = R[64 + p]
    wm0[7, 64:] = L[64 + p]
    wmbig = np.zeros((128, 512), np.float32)
    for gp in (0, 32, 64):
        wmbig[gp:gp + 4, :] = wm4
    wmbig[96:104, 0:128] = wm0
    return {
        "xt_rot": np.ascontiguousarray(x_rot.T),  # [in=256, node=512]
        "cm": cm,
        "wm": wmbig.astype(BF16NP),
    }


# --------------------------------------------------------------------------
# device program
# --------------------------------------------------------------------------

_PROGRAM = None


def _build_program() -> bass.Bass:
    nc = bacc.Bacc()
    f32 = F32
    npad = 68  # padded flat scratch rows

    # ---- dram tensors -----------------------------------------------------
    xt_rot = nc.dram_tensor("xt_rot", [NIN, N], f32, kind="ExternalInput")
    w12t = nc.dram_tensor("w12t", [NIN, 2 * CH], f32, kind="ExternalInput")
    b2_row = nc.dram_tensor("b2_row", [1, 2 * CH], f32, kind="ExternalInput")
    cm = nc.dram_tensor("cm", [128, 8], f32, kind="ExternalInput")
    d_wm = nc.dram_tensor("wm", [128, N], BF16, kind="ExternalInput")
    d_dl = nc.dram_tensor("diag_l", [128, 32 * 128], BF16, kind="ExternalInput")
    d_dr = nc.dram_tensor("diag_r", [128, 32 * 128], BF16, kind="ExternalInput")

    # DMA-native contiguous layouts; the host unpicks them (free).
    # slab_m[3g + J-1, p, (sub, q, ch)] = value(row 8g+2sub+q, s = 128J+p)
    # out0d/u[g, q*64+s, (sub, ch)]    = diag/upper block values
    slab_m = nc.dram_tensor("slab_m", [24, 128, 2048], f32,
                            kind="ExternalOutput")
    out0d = nc.dram_tensor("out0d", [8, 128, 1024], f32, kind="ExternalOutput")
    out0u = nc.dram_tensor("out0u", [8, 128, 1024], f32, kind="ExternalOutput")

    with tile.TileContext(nc) as tc:
        with (
            tc.tile_pool(name="const", bufs=1) as cpool,
            tc.tile_pool(name="tmp", bufs=3) as tpool,
            tc.tile_pool(name="psA", bufs=4, space="PSUM") as psA,
            tc.tile_pool(name="ps0", bufs=4, space="PSUM") as ps0,
            tc.tile_pool(name="stM", bufs=6) as stM,
            tc.tile_pool(name="st0", bufs=4) as st0,
        ):
            # ---- load inputs ---------------------------------------------
            def load(dram, shape, dtype, tag):
                t = cpool.tile(shape, dtype, tag=tag)
                nc.sync.dma_start(out=t[:], in_=dram[:])
                return t

            xt0 = load(xt_rot[0:128, :], [128, N], f32, "xt0")
            xt1 = load(xt_rot[128:256, :], [128, N], f32, "xt1")
            w12a = load(w12t[0:128, :], [128, 2 * CH], f32, "w12a")
            w12b = load(w12t[128:256, :], [128, 2 * CH], f32, "w12b")
            b2t = load(b2_row, [1, 2 * CH], f32, "b2t")
            cmt = load(cm, [128, 8], f32, "cmt")
            wmt = load(d_wm, [128, N], BF16, "wmt")
            dlt = cpool.tile([128, 32 * 128], BF16, tag="dlt")
            nc.gpsimd.dma_start(out=dlt[:], in_=d_dl[:])
            drt = cpool.tile([128, 32 * 128], BF16, tag="drt")
            nc.gpsimd.dma_start(out=drt[:], in_=d_dr[:])

            ones1 = cpool.tile([1, 128], f32, tag="ones1")
            nc.vector.memset(ones1[:], 1.0)

            # ---- phase 1: tables [A | B'] (one [128, 512] psum per s) ----
            A_t, Bp_t = [], []
            for s in range(4):
                pa = psA.tile([128, 2 * CH], f32, tag="pj", name=f"ptb{s}")
                mmd = nc.tensor.matmul
                mmd(pa[:], xt0[:, 128 * s:128 * (s + 1)], w12a[:],
                    start=True, stop=False)
                mmd(pa[:], xt1[:, 128 * s:128 * (s + 1)], w12b[:],
                    start=False, stop=False)
                mmd(pa[:], ones1[:], b2t[:], start=False, stop=True)
                comb = cpool.tile([128, 2 * CH], f32, tag=f"AB{s}")
                if s % 2 == 0:
                    nc.vector.tensor_copy(out=comb[:], in_=pa[:])
                else:
                    nc.scalar.copy(out=comb[:], in_=pa[:])
                A_t.append(comb[:, 0:CH])
                Bp_t.append(comb[:, CH:2 * CH])

            # ---- phase 1b: mixed column tables Cmix = R*B' + L*A (f32) ---
            Cmix = []
            for s in range(4):
                eng = nc.vector if s % 2 == 0 else nc.gpsimd
                t1 = tpool.tile([128, CH], f32, tag="t1")
                eng.tensor_scalar(t1[:], Bp_t[s], cmt[:, s:s + 1], None,
                                  mybir.AluOpType.mult)
                t2 = tpool.tile([128, CH], f32, tag="t2")
                eng.tensor_scalar(t2[:], A_t[s], cmt[:, 4 + s:5 + s], None,
                                  mybir.AluOpType.mult)
                cx = cpool.tile([128, CH], f32, tag=f"C{s}")
                eng.tensor_tensor(cx[:], t1[:], t2[:], mybir.AluOpType.add)
                Cmix.append(cx)

            # duplicated f32 column tables for the r-paired main tiles
            CD = {}
            for s in (1, 2, 3):
                dup = cpool.tile([128, 2 * CH], f32, tag=f"CD{s}")
                nc.vector.tensor_copy(out=dup[:, 0:CH], in_=Cmix[s][:])
                nc.scalar.copy(out=dup[:, CH:2 * CH], in_=Cmix[s][:])
                CD[s] = dup


            def hi_lo(src_ap, tag):
                """split a f32 [128, W] AP into bf16 hi + lo tiles."""
                wdt = src_ap.shape[-1]
                hi = cpool.tile([128, wdt], BF16, tag=f"{tag}h")
                nc.scalar.copy(out=hi[:], in_=src_ap)
                h32 = tpool.tile([128, wdt], f32, tag="h32")
                nc.scalar.copy(out=h32[:], in_=hi[:])
                d = tpool.tile([128, wdt], f32, tag="d32")
                nc.vector.tensor_sub(d[:], src_ap, h32[:])
                lo = cpool.tile([128, wdt], BF16, tag=f"{tag}l")
                nc.vector.tensor_copy(out=lo[:], in_=d[:])
                return hi, lo

            ah, al = hi_lo(A_t[0], "a0")
            bh, bl = hi_lo(Bp_t[0], "b0")
            # block0-upper f32 column table, replicated to both q-halves
            cup = cpool.tile([128, CH], f32, tag="cup")
            nc.gpsimd.dma_start(out=cup[0:64, :], in_=Cmix[0][64:128, :])
            nc.gpsimd.dma_start(out=cup[64:128, :], in_=Cmix[0][64:128, :])
            # diag combined rhs [A_rot[0:64] ; B'_rot[0:64]] (hi / lo)
            dcb_h = cpool.tile([128, CH], BF16, tag="dcbh")
            dcb_l = cpool.tile([128, CH], BF16, tag="dcbl")
            nc.vector.tensor_copy(out=dcb_h[0:64, :], in_=ah[0:64, :])
            nc.vector.tensor_copy(out=dcb_l[0:64, :], in_=al[0:64, :])
            nc.gpsimd.dma_start(out=dcb_h[64:128, :], in_=bh[0:64, :])
            nc.gpsimd.dma_start(out=dcb_l[64:128, :], in_=bl[0:64, :])
            # flat row tables: direct SBUF->SBUF flatten into partitions
            # 0-7, then replicated to partition groups 32/64/96 (walrus
            # requires rhs to start at the same partition as the weights).
            rp4 = cpool.tile([104, 64 * CH], BF16, tag="rp4")
            nc.vector.memset(rp4[0:8, 63 * CH:64 * CH], 0.0)
            for i, t in enumerate((ah, bh, al, bl)):
                nc.sync.dma_start(out=rp4[i:i + 1, :], in_=t[0:64, :])
                nc.sync.dma_start(out=rp4[4 + i:5 + i, 0:63 * CH],
                                  in_=t[1:64, :])
            for gp in (32, 64, 96):
                nc.gpsimd.dma_start(out=rp4[gp:gp + 8, :], in_=rp4[0:8, :])

            # ---- phase 2: main loop --------------------------------------
            for g in range(8):
                sM = {J: stM.tile([128, 4 * 512], f32, tag="sm",
                                  name=f"sm_{g}_{J}")
                      for J in (1, 2, 3)}
                s0d = st0.tile([128, 4 * CH], f32, tag="s0")
                s0u = st0.tile([128, 4 * CH], f32, tag="s0")
                for sub in range(4):
                    rp = 4 * g + sub
                    off = 2 * rp * CH
                    # four small-K masked-broadcast matmuls in distinct PE
                    # row-groups -> concurrent execution.
                    pj = {}
                    for J in (1, 2, 3):
                        gp = 32 * (J - 1)
                        p = psA.tile([128, 512], f32, tag="pj",
                                     name=f"pj_{rp}_{J}")
                        nc.tensor.matmul(
                            p[:], wmt[gp:gp + 4, 128 * J:128 * (J + 1)],
                            rp4[gp:gp + 4, off:off + 512],
                            start=True, stop=True, tile_position=(gp, 0))
                        pj[J] = p
                    pu = ps0.tile([128, CH], f32, tag="p0", name=f"pu_{rp}")
                    nc.tensor.matmul(
                        pu[:], wmt[96:104, 0:128],
                        rp4[96:104, off:off + CH],
                        start=True, stop=True, tile_position=(96, 0))
                    # diagonal block (s in [0,64)), rows r0, r0+1
                    pd = ps0.tile([128, CH], f32, tag="p0", name=f"pd_{rp}")
                    dl_sl = dlt[:, 128 * rp:128 * (rp + 1)]
                    dr_sl = drt[:, 128 * rp:128 * (rp + 1)]
                    nc.tensor.matmul(pd[:], dl_sl, dcb_h[:],
                                     start=True, stop=False)
                    nc.tensor.matmul(pd[:], dl_sl, dcb_l[:],
                                     start=False, stop=False)
                    nc.tensor.matmul(pd[:], dr_sl, dcb_h[:],
                                     start=False, stop=False)
                    nc.tensor.matmul(pd[:], dr_sl, dcb_l[:],
                                     start=False, stop=True)
                    # evacuation: VectorE folds the f32 column tables in;
                    # ScalarE evacuates the diagonal block.
                    for J in (1, 2, 3):
                        nc.vector.tensor_add(
                            sM[J][:, 512 * sub:512 * (sub + 1)],
                            pj[J][:], CD[J][:])
                    nc.vector.tensor_add(
                        s0u[:, CH * sub:CH * (sub + 1)], pu[:], cup[:])
                    nc.scalar.copy(out=s0d[:, CH * sub:CH * (sub + 1)],
                                   in_=pd[:])
                    if sub in (1, 3):
                        h = (sub - 1) // 2
                        hs, he = 1024 * h, 1024 * (h + 1)
                        for J in (1, 2):
                            nc.sync.dma_start(
                                out=slab_m[3 * g + J - 1][:, hs:he],
                                in_=sM[J][:, hs:he])
                        nc.scalar.dma_start(
                            out=slab_m[3 * g + 2][:, hs:he],
                            in_=sM[3][:, hs:he])
                        nc.scalar.dma_start(
                            out=out0u[g][:, 512 * h:512 * (h + 1)],
                            in_=s0u[:, 512 * h:512 * (h + 1)])
                        nc.scalar.dma_start(
                            out=out0d[g][:, 512 * h:512 * (h + 1)],
                            in_=s0d[:, 512 * h:512 * (h + 1)])

    nc.compile()
    return nc


def _program() -> bass.Bass:
    global _PROGRAM
    if _PROGRAM is None:
        _PROGRAM = _build_program()
    return _PROGRAM


# --------------------------------------------------------------------------
# host entry point
# --------------------------------------------------------------------------

def _assemble(results):
    """8 per-core result dicts -> full [512, 512, 256] output."""
    out = np.empty((N, N, CH), np.float32)
    for k in range(NCORES):
        r = results[k]
        slab = np.empty((RB, N, CH), np.float32)
        # out0d/u: [g, q*64+s, (sub, ch)] -> rows 8g+2sub+q, cols s / 64+s
        d = np.asarray(r["out0d"]).reshape(8, 2, 64, 4, CH)
        slab[:, 0:64, :] = d.transpose(0, 3, 1, 2, 4).reshape(RB, 64, CH)
        u = np.asarray(r["out0u"]).reshape(8, 2, 64, 4, CH)
        slab[:, 64:128, :] = u.transpose(0, 3, 1, 2, 4).reshape(RB, 64, CH)
        # slab_m: [3g+J-1, p, (sub, q, ch)] -> rows 8g+2sub+q, col 128J+p
        m = np.asarray(r["slab_m"]).reshape(8, 3, 128, 4, 2, CH)
        slab[:, 128:512, :] = (
            m.transpose(0, 3, 4, 1, 2, 5).reshape(RB, 384, CH))
        base = RB * k
        out[base:base + RB] = np.roll(slab, base, axis=1)
    return out


def build_in_maps(x, W, b):
    shared = _shared_inputs(W, b)
    return [dict(shared, **_core_inputs(x, k)) for k in range(NCORES)]


def kernel(x, W, b):
    nc = _program()
    in_maps = build_in_maps(x, W, b)
    res = run_bass_kernel_spmd(nc, in_maps, core_ids=list(range(NCORES)))
    return _assemble(res.results)
